# revision 21
# baseline (speedup 1.0000x reference)
"""Multi-head attention Trainium2 kernel (8 NeuronCores, SPMD), v4.

Problem: B=4, T=2048, n_feat=512, H=8 heads, d_k=64.
Sharding: core c -> batch b = c//2, head-half hh = c%2 (4 heads = 256 attn dims).

Design (ACT-exp-bound: 128 exps of [128,1024] = ~133us on the scalar engine is
the floor; everything else hides under it):
- Host pre-transposes + bf16-casts activations (x^T [512,2048]) and weights.
- Q^T/K^T projections in [o, t] layout, V in [t, o] layout with a ones column
  (softmax denominator falls out of the PV matmul).
- Scores S^T[j, i] per (head, i-super of 1024); exp on ACT, bf16 out.
- PV out[i-block(128), dk+1] with et stationary. TRN2 psum accumulation must
  be one contiguous run per region (interleaved/reopened groups silently drop
  a visit), so PV for pair p runs ib-major (16 consecutive matmuls per
  region) interleaved into pair p+1's jt loop; all 16 et tiles of a pair stay
  live in SBUF.
- Norm = reciprocal + per-partition scalar mul (denominator is psum col 64).
- Normalized x transposed on PE in 64x64 quadrants (tile_position) so both
  heads of a pair-half pack onto 128 partitions -> output projection
  contracts 2x128 instead of 4x64 (half the matmuls).
- Tail: last pair's PV split into jt-halves (lo into xpa/xpb, hi into pp-tag
  psum), norm/transpose/outproj pipelined per i-half, ACT helps after the exp
  stream ends. Head: packed DMA prefix + PE warmup to beat the pstate ramp.
Host sums the two head-half partials per batch, transposes, adds bo.
"""
import sys

sys.path.insert(0, "/opt/trn_rl_repo")

import numpy as np
import ml_dtypes

import concourse.bass as bass
import concourse.tile as tile
from concourse import bacc, mybir
from concourse.bass_utils import run_bass_kernel_spmd

P = 128
T = 2048
F = 512            # n_feat (projection contraction dim)
OB = 256           # per-core attention dims (4 heads x 64)
NH = 4             # local heads
DK = 64
NT = T // P        # 16 row tiles
FO = F // P        # 4 feature tiles
NSUP = 2           # i-supers per head
ISUP = T // NSUP   # 1024
NIB = ISUP // P    # 8 i-blocks per super
JT = NT            # 16 j tiles
EPS = 1e-8
D1 = DK + 1

f32 = mybir.dt.float32
bf16 = mybir.dt.bfloat16
BF = ml_dtypes.bfloat16
Exp = mybir.ActivationFunctionType.Exp

_CACHE = {}


def _build(mask_ones: bool):
    nc = bacc.Bacc("TRN2", target_bir_lowering=False, debug=False, num_devices=8)

    xk = nc.dram_tensor("xk", (F, T), bf16, kind="ExternalInput").ap()
    xq = nc.dram_tensor("xq", (F, T), bf16, kind="ExternalInput").ap()
    xv = nc.dram_tensor("xv", (F, T), bf16, kind="ExternalInput").ap()
    wkq = nc.dram_tensor("wkq", (2, F, OB), bf16, kind="ExternalInput").ap()
    wvd = nc.dram_tensor("wvd", (F, OB), bf16, kind="ExternalInput").ap()
    wo = nc.dram_tensor("wo", (OB, F), bf16, kind="ExternalInput").ap()
    bqk = nc.dram_tensor("bqk", (P, 4), f32, kind="ExternalInput").ap()
    bvb = nc.dram_tensor("bvb", (P, OB), f32, kind="ExternalInput").ap()
    ident = nc.dram_tensor("ident", (P, P), bf16, kind="ExternalInput").ap()
    if not mask_ones:
        mcol = nc.dram_tensor("mcol", (P, NT), f32, kind="ExternalInput").ap()
    outT = nc.dram_tensor("outT", (F, T), bf16, kind="ExternalOutput").ap()

    xk_r = xk.rearrange("(fo p) t -> p fo t", p=P)
    xq_r = xq.rearrange("(fo p) t -> p fo t", p=P)
    xv_r = xv.rearrange("(fo p) t -> p fo t", p=P)

    with tile.TileContext(nc) as tc:
        with tc.tile_pool(name="const", bufs=1) as cpool, \
             tc.tile_pool(name="act", bufs=1) as apool, \
             tc.tile_pool(name="persist", bufs=1) as ppool, \
             tc.tile_pool(name="et", bufs=26) as epool, \
             tc.tile_pool(name="norm", bufs=2) as npool, \
             tc.tile_pool(name="out", bufs=4) as opool, \
             tc.tile_pool(name="ps_st", bufs=2, space="PSUM") as ps_st, \
             tc.tile_pool(name="ps_pp", bufs=2, space="PSUM") as ps_pp, \
             tc.tile_pool(name="ps_xp", bufs=1, space="PSUM") as ps_xp:

            # ---- SBUF tiles ----
            wkqv_sb = cpool.tile([P, 3, FO, OB], bf16, tag="wkqv")
            wo_sb = cpool.tile([P, 2, F], bf16, tag="wo")
            bqk_sb = cpool.tile([P, 4], f32, tag="bqk")
            bv_sb = cpool.tile([P, OB], f32, tag="bv")
            id_sb = cpool.tile([P, P], bf16, tag="ident")
            ws = cpool.tile([P, OB], bf16, tag="warm")
            if not mask_ones:
                mc_sb = cpool.tile([P, NT], f32, tag="mcol")
                z3_sb = cpool.tile([P, NH, 1], f32, tag="z3")

            xk_sb = apool.tile([P, FO, T], bf16, tag="xk")
            xq_sb = apool.tile([P, FO, T], bf16, tag="xq")
            xv_sb = apool.tile([P, FO, T], bf16, tag="xv")

            KT = ppool.tile([P, OB // P, T], bf16, tag="KT")
            QT = ppool.tile([P, OB // P, T], bf16, tag="QT")
            V2 = ppool.tile([P, NT, NH, D1], bf16, tag="V2")
            xT2 = ppool.tile([P, 2, T], bf16, tag="xT2")

            wk_sb = wkqv_sb[:, 0, :, :]
            wq_sb = wkqv_sb[:, 1, :, :]
            wv_sb = wkqv_sb[:, 2, :, :]
            bk_col = bqk_sb[:, 0:2]
            bq_col = bqk_sb[:, 2:4]

            # ---- PE warmup: keep PE busy through the pstate ramp ----
            nc.vector.memset(ws[:], 0.01)
            for wi in range(26):
                wp = ps_st.tile([P, OB], f32, tag="st", name=f"warm_{wi}")
                nc.tensor.matmul(
                    wp[:, :OB], ws[:, 0:P], ws[:, 0:OB], start=True, stop=True
                )

            # ---- DMA plan (single SP queue, prioritized order) ----
            def dma_x(xr, dst, c, lo=0, hi=F):
                nc.sync.dma_start(
                    out=dst[:, :, c * F + lo:c * F + hi],
                    in_=xr[:, :, c * F + lo:c * F + hi],
                )

            nc.scalar.dma_start(out=bqk_sb[:], in_=bqk[:])
            nc.scalar.dma_start(out=bv_sb[:], in_=bvb[:])
            nc.scalar.dma_start(out=id_sb[:], in_=ident[:])
            if not mask_ones:
                nc.scalar.dma_start(out=mc_sb[:], in_=mcol[:])
            nc.sync.dma_start(
                out=wkqv_sb[:, 0:2, :, :],
                in_=wkq.rearrange("w (fo p) o -> p w fo o", p=P),
            )
            dma_x(xk_r, xk_sb, 0)
            dma_x(xq_r, xq_sb, 0)
            dma_x(xq_r, xq_sb, 1)
            nc.sync.dma_start(
                out=wkqv_sb[:, 2, :, :],
                in_=wvd.rearrange("(fo p) o -> p fo o", p=P),
            )
            dma_x(xv_r, xv_sb, 0)
            dma_x(xk_r, xk_sb, 1)
            dma_x(xv_r, xv_sb, 1)
            dma_x(xk_r, xk_sb, 2)
            dma_x(xv_r, xv_sb, 2)
            dma_x(xk_r, xk_sb, 3)
            dma_x(xv_r, xv_sb, 3)
            dma_x(xq_r, xq_sb, 2)
            dma_x(xq_r, xq_sb, 3)
            nc.sync.dma_start(out=wo_sb[:], in_=wo.rearrange("(oh p) f -> p oh f", p=P))

            # V2 ones column (or mask column)
            nc.vector.memset(V2[:, :, :, DK:D1], 1.0)
            if not mask_ones:
                nc.vector.memset(z3_sb[:], 0.0)

            # ---- projection emitters ----
            def qk_chunk(w_sb, b_col, dst, x_sb, po, c, lo=0, hi=F):
                w = hi - lo
                pp = ps_pp.tile([P, F], f32, tag="pp",
                                name=f"qk_{po}_{c}_{lo}")
                for fo in range(FO):
                    nc.tensor.matmul(
                        pp[:, 0:w],
                        w_sb[:, fo, po * P:(po + 1) * P],
                        x_sb[:, fo, c * F + lo:c * F + hi],
                        start=(fo == 0),
                        stop=(fo == FO - 1),
                    )
                nc.vector.tensor_scalar_add(
                    dst[:, po, c * F + lo:c * F + hi], pp[:, 0:w],
                    b_col[:, po:po + 1]
                )

            def v_chunk(tb):
                pp = ps_pp.tile([P, F], f32, tag="pp", name=f"v_{tb}")
                for fo in range(FO):
                    nc.tensor.matmul(
                        pp[:, :OB],
                        xv_sb[:, fo, tb * P:(tb + 1) * P],
                        wv_sb[:, fo, :],
                        start=(fo == 0),
                        stop=(fo == FO - 1),
                    )
                if mask_ones:
                    nc.vector.tensor_add(
                        V2[:, tb, :, 0:DK],
                        pp[:, :OB].rearrange("p (h d) -> p h d", h=NH),
                        bv_sb[:].rearrange("p (h d) -> p h d", h=NH),
                    )
                else:
                    vt = npool.tile([P, NH, DK], f32, tag="vt")
                    nc.vector.tensor_add(
                        vt[:],
                        pp[:, :OB].rearrange("p (h d) -> p h d", h=NH),
                        bv_sb[:].rearrange("p (h d) -> p h d", h=NH),
                    )
                    nc.vector.tensor_scalar_mul(
                        V2[:, tb, :, 0:DK], vt[:], mc_sb[:, tb:tb + 1]
                    )
                    nc.vector.tensor_scalar_add(
                        V2[:, tb, :, DK:D1], z3_sb[:], mc_sb[:, tb:tb + 1]
                    )

            # ---- head start: K po0 c0, Q po0 c0-c1 ----
            qk_chunk(wk_sb, bk_col, KT, xk_sb, 0, 0)
            qk_chunk(wq_sb, bq_col, QT, xq_sb, 0, 0)

            # ---- norm / transpose / outproj emitters ----
            def emit_norm_mul(xn, ib, src_ap, rz, eng_act=False):
                if eng_act:
                    nc.scalar.mul(xn[:, ib, :], src_ap, rz[:, ib:ib + 1])
                else:
                    nc.vector.tensor_scalar_mul(xn[:, ib, :], src_ap,
                                                rz[:, ib:ib + 1])

            xn2_hold = {}

            def get_xn2(oh, su):
                if (oh, su) not in xn2_hold:
                    xn2_hold[(oh, su)] = npool.tile(
                        [P, NIB, 2, DK], bf16, tag="xn",
                        name=f"xn2_{oh}_{su}")
                return xn2_hold[(oh, su)]

            def emit_norm(pi, h, su, xpa, xpb):
                zeps = npool.tile([P, NIB], f32, tag="zeps")
                za = xpa[:].rearrange("p (ib c) -> p ib c", c=D1)[
                    :, :, DK:D1].rearrange("p ib one -> p (ib one)")
                zb = xpb[:].rearrange("p (ib c) -> p ib c", c=D1)[
                    :, :, DK:D1].rearrange("p ib one -> p (ib one)")
                nc.vector.tensor_scalar_add(zeps[:, 0:4], za, EPS)
                nc.vector.tensor_scalar_add(zeps[:, 4:8], zb, EPS)
                rz = npool.tile([P, NIB], f32, tag="rz")
                nc.vector.reciprocal(rz[:], zeps[:])
                xn2 = get_xn2(h // 2, su)
                for ib in range(NIB):
                    src = xpa if ib < 4 else xpb
                    ib2 = ib % 4
                    nc.vector.tensor_scalar_mul(
                        xn2[:, ib, h % 2, :],
                        src[:, ib2 * D1:ib2 * D1 + DK],
                        rz[:, ib:ib + 1],
                    )
                return xn2

            def emit_tp_ib(tp, xn2, ib):
                # full 128x128 transpose: [i, (h d)] -> [(h d), i]
                nc.tensor.transpose(
                    tp[:, ib * P:(ib + 1) * P],
                    xn2[:, ib, :, :].rearrange("p h d -> p (h d)"),
                    id_sb[:],
                )

            def emit_tp(oh, su, xn2, ib_lo=0, nib=NIB):
                tp = ps_pp.tile([P, ISUP], bf16, tag="pp",
                                name=f"tp_{oh}_{su}_{ib_lo}")
                for ib in range(ib_lo, ib_lo + nib):
                    emit_tp_ib(tp, xn2, ib)
                nc.vector.tensor_copy(
                    xT2[:, oh, su * ISUP + ib_lo * P:
                        su * ISUP + (ib_lo + nib) * P],
                    tp[:, ib_lo * P:(ib_lo + nib) * P],
                )

            os2_hold = {}

            def op_group(m2, su, cc, eng_act=False, dma_each=False):
                if cc == 0:
                    os2_hold[m2] = opool.tile([P, 2, F], bf16, tag="os2",
                                              name=f"os2_{su}_{m2}")
                os2 = os2_hold[m2]
                c = su * 2 + cc
                pp = ps_pp.tile([P, F], f32, tag="pp", name=f"op_{m2}_{c}")
                for oh in range(2):
                    nc.tensor.matmul(
                        pp[:, :F],
                        wo_sb[:, oh, m2 * P:(m2 + 1) * P],
                        xT2[:, oh, c * F:(c + 1) * F],
                        start=(oh == 0),
                        stop=(oh == 1),
                    )
                if eng_act:
                    nc.scalar.copy(os2[:, cc, :], pp[:, :F])
                else:
                    nc.vector.tensor_copy(os2[:, cc, :], pp[:, :F])
                if dma_each:
                    nc.sync.dma_start(
                        out=outT[m2 * P:(m2 + 1) * P, c * F:(c + 1) * F],
                        in_=os2[:, cc, :],
                    )
                elif cc == 1:
                    nc.sync.dma_start(
                        out=outT[m2 * P:(m2 + 1) * P, su * ISUP:(su + 1) * ISUP],
                        in_=os2[:].rearrange("p c f -> p (c f)"),
                    )

            # ---- per-pair interleaved extras ----
            def K_(po, c):
                return lambda: qk_chunk(wk_sb, bk_col, KT, xk_sb, po, c)

            def Q_(po, c):
                return lambda: qk_chunk(wq_sb, bq_col, QT, xq_sb, po, c)

            def OP_(m2, su, cc):
                return lambda: op_group(m2, su, cc)

            extras_by_pair = {
                0: {1: [K_(0, 1)], 2: [K_(0, 2)], 3: [K_(0, 3)]},
                1: {9: [K_(1, 0)], 10: [K_(1, 1)], 11: [K_(1, 2)],
                    12: [K_(1, 3)], 13: [Q_(1, 0)], 14: [Q_(1, 1)]},
                2: {8: [Q_(1, 2)], 10: [Q_(1, 3)], 12: [Q_(0, 2)],
                    14: [Q_(0, 3)]},
                5: {3: [OP_(0, 0, 0)], 5: [OP_(0, 0, 1)], 7: [OP_(1, 0, 0)],
                    9: [OP_(1, 0, 1)]},
                6: {1: [OP_(2, 0, 0)], 3: [OP_(2, 0, 1)], 5: [OP_(3, 0, 0)],
                    7: [OP_(3, 0, 1)]},
            }

            # ---- attention pairs ----
            # PV for pair p: 8 ib-major runs (16 consecutive matmuls each)
            # interleaved into pair p+1's jt loop. Last pair: jt-halves.
            pairs = [(su, h) for su in range(NSUP) for h in range(NH)]
            LAST = len(pairs) - 1
            LOJT = 10
            LO_SCHED = [[0, 1], [2, 3], [4], [5], [6], [7]]
            state = {}

            def pv_run(p, ib, jt0=0, njt=JT, tgt_pair=None, fresh=True):
                ps = state[p]
                if tgt_pair is None:
                    if ib == 0 and fresh:
                        ps["xpa"] = ps_xp.tile([P, 4 * D1], f32, tag="xpa",
                                               name=f"xpa_{p}")
                        ps["xpb"] = ps_xp.tile([P, 4 * D1], f32, tag="xpb",
                                               name=f"xpb_{p}")
                    tgt = ps["xpa"] if ib < 4 else ps["xpb"]
                else:
                    tgt = tgt_pair[0] if ib < 4 else tgt_pair[1]
                ib2 = ib % 4
                hp = ps["h"]
                for jj in range(njt):
                    jt = jt0 + jj
                    nc.tensor.matmul(
                        tgt[:, ib2 * D1:(ib2 + 1) * D1],
                        ps["et"][jt][:, ib * P:(ib + 1) * P],
                        V2[:, jt, hp, :],
                        start=(jj == 0),
                        stop=(jj == njt - 1),
                    )

            def drain_pair(p):
                ps = state[p]
                h, su = ps["h"], ps["su"]
                xn2 = emit_norm(p, h, su, ps["xpa"], ps["xpb"])
                if h % 2 == 1:
                    emit_tp(h // 2, su, xn2)
                    del xn2_hold[(h // 2, su)]
                del state[p]

            for pi, (su, h) in enumerate(pairs):
                qoff = (h % 2) * DK
                qpo = h // 2
                isl = su * ISUP
                extras = extras_by_pair.get(pi, {})
                state[pi] = {"h": h, "su": su, "et": []}

                def scores(jt):
                    st = ps_st.tile([P, ISUP], f32, tag="st",
                                    name=f"st_{pi}_{jt}")
                    for c2 in range(ISUP // F):
                        nc.tensor.matmul(
                            st[:, c2 * F:(c2 + 1) * F],
                            KT[qoff:qoff + DK, qpo, jt * P:(jt + 1) * P],
                            QT[qoff:qoff + DK, qpo,
                               isl + c2 * F:isl + (c2 + 1) * F],
                            start=True,
                            stop=True,
                        )
                    return st

                H2 = F // 2

                def st0_part(st, lo, hi):
                    nc.tensor.matmul(
                        st[:, lo:hi],
                        KT[qoff:qoff + DK, qpo, 0:P],
                        QT[qoff:qoff + DK, qpo, isl + lo:isl + hi],
                        start=True, stop=True,
                    )

                if pi == 0:
                    # first scores tile built in halves, so the exp stream
                    # starts as soon as the first QT half lands
                    st_prev = ps_st.tile([P, ISUP], f32, tag="st",
                                         name="st_0_0")
                    st0_part(st_prev, 0, F)
                else:
                    st_prev = scores(0)
                for jt in range(JT):
                    et = epool.tile([P, ISUP], bf16, tag="et",
                                    name=f"et_{pi}_{jt}")
                    if pi == 0 and jt == 0:
                        nc.scalar.activation(et[:, 0:F], st_prev[:, 0:F],
                                             Exp, scale=0.125)
                        qk_chunk(wq_sb, bq_col, QT, xq_sb, 0, 1)
                        st0_part(st_prev, F, ISUP)
                        nc.scalar.activation(et[:, F:ISUP], st_prev[:, F:ISUP],
                                             Exp, scale=0.125)
                    else:
                        nc.scalar.activation(et[:], st_prev[:], Exp,
                                             scale=0.125)
                    state[pi]["et"].append(et)
                    if jt + 1 < JT:
                        st_prev = scores(jt + 1)
                    if pi == 0:
                        v_chunk(jt)
                    if pi > 0 and jt < NIB:
                        pv_run(pi - 1, jt)
                    if pi > 0 and jt == NIB:
                        drain_pair(pi - 1)
                    if pi == LAST and jt >= 10:
                        # lo runs (jts 0-9) spread over jts 10-15
                        for ibx in LO_SCHED[jt - 10]:
                            pv_run(LAST, ibx, jt0=0, njt=LOJT,
                                   fresh=(ibx == 0))
                    for fn in extras.get(jt, []):
                        fn()

            # ---- tail: last pair hi-half + norm/tp/outproj per i-half ----
            ps7 = state[LAST]
            h7, su7 = ps7["h"], ps7["su"]
            # lo halves -> SBUF (overlaps the hi runs; DVE can read only one
            # PSUM operand per tensor_tensor)
            lo_sb = npool.tile([P, NIB * D1], f32, tag="losb")
            nc.vector.tensor_copy(lo_sb[:, 0:4 * D1], ps7["xpa"][:])
            nc.vector.tensor_copy(lo_sb[:, 4 * D1:NIB * D1], ps7["xpb"][:])
            hia = ps_pp.tile([P, 4 * D1], f32, tag="pp", name="hia")
            hib = ps_pp.tile([P, 4 * D1], f32, tag="pp", name="hib")
            for ib in range(NIB):
                pv_run(LAST, ib, jt0=LOJT, njt=JT - LOJT,
                       tgt_pair=(hia, hib))

            def z_of(xp):
                return xp[:].rearrange("p (ib c) -> p ib c", c=D1)[
                    :, :, DK:D1].rearrange("p ib one -> p (ib one)")

            # norm: z = (z_lo + eps) + z_hi fused; muls split DVE/ACT
            zs = npool.tile([P, NIB], f32, tag="zeps")
            nc.vector.scalar_tensor_tensor(
                out=zs[:, 0:4], in0=z_of(lo_sb)[:, 0:4], scalar=EPS,
                in1=z_of(hia), op0=mybir.AluOpType.add,
                op1=mybir.AluOpType.add)
            nc.vector.scalar_tensor_tensor(
                out=zs[:, 4:8], in0=z_of(lo_sb)[:, 4:8], scalar=EPS,
                in1=z_of(hib), op0=mybir.AluOpType.add,
                op1=mybir.AluOpType.add)
            rz7 = npool.tile([P, NIB], f32, tag="rz")
            nc.vector.reciprocal(rz7[:], zs[:])
            xn27 = get_xn2(h7 // 2, su7)
            xs7 = npool.tile([P, NIB, DK], f32, tag="xs7")

            def norm_half(ib_lo):
                hi = hia if ib_lo < 4 else hib
                lo4 = lo_sb[:].rearrange("p (ib c) -> p ib c", c=D1)[
                    :, ib_lo:ib_lo + 4, 0:DK]
                hi4 = hi[:].rearrange("p (ib c) -> p ib c", c=D1)[
                    :, :, 0:DK]
                nc.vector.tensor_add(xs7[:, ib_lo:ib_lo + 4, :], lo4, hi4)
                for ib in range(ib_lo, ib_lo + 4):
                    if ib % 2 == 1:
                        nc.scalar.mul(xn27[:, ib, h7 % 2, :], xs7[:, ib, :],
                                      rz7[:, ib:ib + 1])
                    else:
                        nc.vector.tensor_scalar_mul(
                            xn27[:, ib, h7 % 2, :], xs7[:, ib, :],
                            rz7[:, ib:ib + 1])

            norm_half(0)
            norm_half(4)
            emit_tp(h7 // 2, su7, xn27, ib_lo=0, nib=4)
            for m2 in range(F // P):
                op_group(m2, 1, 0, eng_act=(m2 % 2 == 1), dma_each=True)
            emit_tp(h7 // 2, su7, xn27, ib_lo=4, nib=4)
            for m2 in range(F // P):
                op_group(m2, 1, 1, eng_act=(m2 % 2 == 1), dma_each=True)

    nc.compile()
    return nc


def _prep_in_maps(query, key, value, mask, Wq, bq, Wk, bk, Wv, bv, Wo,
                  mask_ones):
    ident = np.eye(P, dtype=np.float32).astype(BF)
    B = query.shape[0]
    xTs = {}
    for b in range(B):
        m01 = (mask[b, 0, :] != 0)
        xv_full = value[b] * m01[:, None].astype(np.float32)
        xTs[b] = (
            np.ascontiguousarray(key[b].T).astype(BF),
            np.ascontiguousarray(query[b].T).astype(BF),
            np.ascontiguousarray(xv_full.T).astype(BF),
            np.ascontiguousarray(
                m01.astype(np.float32).reshape(NT, P).T) if not mask_ones
            else None,
        )
    in_maps = []
    for c in range(8):
        b = c // 2
        hh = c % 2
        ob = slice(hh * OB, (hh + 1) * OB)
        xkT, xqT, xvT, mc = xTs[b]
        bqk_h = np.concatenate(
            [bk[ob].reshape(OB // P, P).T, bq[ob].reshape(OB // P, P).T],
            axis=1,
        )
        wkq_h = np.stack([
            np.ascontiguousarray(Wk[ob, :].T),
            np.ascontiguousarray(Wq[ob, :].T),
        ]).astype(BF)
        wv_h = np.ascontiguousarray(Wv[ob, :].T).astype(BF)
        m = {
            "xk": xkT,
            "xq": xqT,
            "xv": xvT,
            "wkq": wkq_h,
            "wvd": wv_h,
            "wo": np.ascontiguousarray(Wo[:, ob].T).astype(BF),
            "bqk": np.ascontiguousarray(bqk_h),
            "bvb": np.ascontiguousarray(np.tile(bv[ob][None, :], (P, 1))),
            "ident": ident,
        }
        if not mask_ones:
            m["mcol"] = mc
        in_maps.append(m)
    return in_maps


def kernel(query, key, value, mask, Wq, bq, Wk, bk, Wv, bv, Wo, bo):
    query = np.asarray(query, dtype=np.float32)
    key = np.asarray(key, dtype=np.float32)
    value = np.asarray(value, dtype=np.float32)
    mask = np.asarray(mask)
    Wq = np.asarray(Wq, dtype=np.float32)
    bq = np.asarray(bq, dtype=np.float32)
    Wk = np.asarray(Wk, dtype=np.float32)
    bk = np.asarray(bk, dtype=np.float32)
    Wv = np.asarray(Wv, dtype=np.float32)
    bv = np.asarray(bv, dtype=np.float32)
    Wo = np.asarray(Wo, dtype=np.float32)
    bo = np.asarray(bo, dtype=np.float32)

    mask_ones = bool(np.all(mask != 0))
    ckey = ("nc", mask_ones)
    if ckey not in _CACHE:
        _CACHE[ckey] = _build(mask_ones)
        _CACHE["nc"] = _CACHE[ckey]  # test.py reads _CACHE["nc"]
    nc = _CACHE[ckey]

    B = query.shape[0]
    in_maps = _prep_in_maps(
        query, key, value, mask, Wq, bq, Wk, bk, Wv, bv, Wo, mask_ones
    )
    res = None
    for attempt in range(3):
        try:
            res = run_bass_kernel_spmd(nc, in_maps, core_ids=list(range(8)))
            break
        except Exception:
            if attempt == 2:
                raise

    out = np.empty((B, T, F), dtype=np.float32)
    for b in range(B):
        acc = (np.asarray(res.results[2 * b]["outT"], dtype=np.float32)
               + np.asarray(res.results[2 * b + 1]["outT"], dtype=np.float32))
        out[b] = acc.T + bo[None, :]
    return out


# revision 22
# speedup vs baseline: 1.0010x; 1.0010x over previous
"""Multi-head attention Trainium2 kernel (8 NeuronCores, SPMD), v4.

Problem: B=4, T=2048, n_feat=512, H=8 heads, d_k=64.
Sharding: core c -> batch b = c//2, head-half hh = c%2 (4 heads = 256 attn dims).

Design (ACT-exp-bound: 128 exps of [128,1024] = ~133us on the scalar engine is
the floor; everything else hides under it):
- Host pre-transposes + bf16-casts activations (x^T [512,2048]) and weights.
- Q^T/K^T projections in [o, t] layout, V in [t, o] layout with a ones column
  (softmax denominator falls out of the PV matmul).
- Scores S^T[j, i] per (head, i-super of 1024); exp on ACT, bf16 out.
- PV out[i-block(128), dk+1] with et stationary. TRN2 psum accumulation must
  be one contiguous run per region (interleaved/reopened groups silently drop
  a visit), so PV for pair p runs ib-major (16 consecutive matmuls per
  region) interleaved into pair p+1's jt loop; all 16 et tiles of a pair stay
  live in SBUF.
- Norm = reciprocal + per-partition scalar mul (denominator is psum col 64).
- Normalized x transposed on PE in 64x64 quadrants (tile_position) so both
  heads of a pair-half pack onto 128 partitions -> output projection
  contracts 2x128 instead of 4x64 (half the matmuls).
- Tail: last pair's PV split into jt-halves (lo into xpa/xpb, hi into pp-tag
  psum), norm/transpose/outproj pipelined per i-half, ACT helps after the exp
  stream ends. Head: packed DMA prefix + PE warmup to beat the pstate ramp.
Host sums the two head-half partials per batch, transposes, adds bo.
"""
import sys

sys.path.insert(0, "/opt/trn_rl_repo")

import numpy as np
import ml_dtypes

import concourse.bass as bass
import concourse.tile as tile
from concourse import bacc, mybir
from concourse.bass_utils import run_bass_kernel_spmd

P = 128
T = 2048
F = 512            # n_feat (projection contraction dim)
OB = 256           # per-core attention dims (4 heads x 64)
NH = 4             # local heads
DK = 64
NT = T // P        # 16 row tiles
FO = F // P        # 4 feature tiles
NSUP = 2           # i-supers per head
ISUP = T // NSUP   # 1024
NIB = ISUP // P    # 8 i-blocks per super
JT = NT            # 16 j tiles
EPS = 1e-8
D1 = DK + 1

f32 = mybir.dt.float32
bf16 = mybir.dt.bfloat16
BF = ml_dtypes.bfloat16
Exp = mybir.ActivationFunctionType.Exp

_CACHE = {}


def _build(mask_ones: bool):
    nc = bacc.Bacc("TRN2", target_bir_lowering=False, debug=False, num_devices=8)

    xk = nc.dram_tensor("xk", (F, T), bf16, kind="ExternalInput").ap()
    xq = nc.dram_tensor("xq", (F, T), bf16, kind="ExternalInput").ap()
    xv = nc.dram_tensor("xv", (F, T), bf16, kind="ExternalInput").ap()
    wkq = nc.dram_tensor("wkq", (2, F, OB), bf16, kind="ExternalInput").ap()
    wvd = nc.dram_tensor("wvd", (F, OB), bf16, kind="ExternalInput").ap()
    wo = nc.dram_tensor("wo", (OB, F), bf16, kind="ExternalInput").ap()
    bqk = nc.dram_tensor("bqk", (P, 4), f32, kind="ExternalInput").ap()
    bvb = nc.dram_tensor("bvb", (P, OB), f32, kind="ExternalInput").ap()
    ident = nc.dram_tensor("ident", (P, P), bf16, kind="ExternalInput").ap()
    if not mask_ones:
        mcol = nc.dram_tensor("mcol", (P, NT), f32, kind="ExternalInput").ap()
    outT = nc.dram_tensor("outT", (F, T), bf16, kind="ExternalOutput").ap()

    xk_r = xk.rearrange("(fo p) t -> p fo t", p=P)
    xq_r = xq.rearrange("(fo p) t -> p fo t", p=P)
    xv_r = xv.rearrange("(fo p) t -> p fo t", p=P)

    with tile.TileContext(nc) as tc:
        with tc.tile_pool(name="const", bufs=1) as cpool, \
             tc.tile_pool(name="act", bufs=1) as apool, \
             tc.tile_pool(name="persist", bufs=1) as ppool, \
             tc.tile_pool(name="et", bufs=26) as epool, \
             tc.tile_pool(name="norm", bufs=2) as npool, \
             tc.tile_pool(name="out", bufs=4) as opool, \
             tc.tile_pool(name="ps_st", bufs=2, space="PSUM") as ps_st, \
             tc.tile_pool(name="ps_pp", bufs=2, space="PSUM") as ps_pp, \
             tc.tile_pool(name="ps_xp", bufs=1, space="PSUM") as ps_xp:

            # ---- SBUF tiles ----
            wkqv_sb = cpool.tile([P, 3, FO, OB], bf16, tag="wkqv")
            wo_sb = cpool.tile([P, 2, F], bf16, tag="wo")
            bqk_sb = cpool.tile([P, 4], f32, tag="bqk")
            bv_sb = cpool.tile([P, OB], f32, tag="bv")
            id_sb = cpool.tile([P, P], bf16, tag="ident")
            ws = cpool.tile([P, OB], bf16, tag="warm")
            if not mask_ones:
                mc_sb = cpool.tile([P, NT], f32, tag="mcol")
                z3_sb = cpool.tile([P, NH, 1], f32, tag="z3")

            xk_sb = apool.tile([P, FO, T], bf16, tag="xk")
            xq_sb = apool.tile([P, FO, T], bf16, tag="xq")
            xv_sb = apool.tile([P, FO, T], bf16, tag="xv")

            KT = ppool.tile([P, OB // P, T], bf16, tag="KT")
            QT = ppool.tile([P, OB // P, T], bf16, tag="QT")
            V2 = ppool.tile([P, NT, NH, D1], bf16, tag="V2")
            xT2 = ppool.tile([P, 2, T], bf16, tag="xT2")

            wk_sb = wkqv_sb[:, 0, :, :]
            wq_sb = wkqv_sb[:, 1, :, :]
            wv_sb = wkqv_sb[:, 2, :, :]
            bk_col = bqk_sb[:, 0:2]
            bq_col = bqk_sb[:, 2:4]

            # ---- PE warmup: keep PE busy through the pstate ramp ----
            nc.vector.memset(ws[:], 0.01)
            for wi in range(30):
                wp = ps_st.tile([P, OB], f32, tag="st", name=f"warm_{wi}")
                nc.tensor.matmul(
                    wp[:, :OB], ws[:, 0:P], ws[:, 0:OB], start=True, stop=True
                )

            # ---- DMA plan (single SP queue, prioritized order) ----
            def dma_x(xr, dst, c, lo=0, hi=F):
                nc.sync.dma_start(
                    out=dst[:, :, c * F + lo:c * F + hi],
                    in_=xr[:, :, c * F + lo:c * F + hi],
                )

            nc.scalar.dma_start(out=bqk_sb[:], in_=bqk[:])
            nc.scalar.dma_start(out=bv_sb[:], in_=bvb[:])
            nc.scalar.dma_start(out=id_sb[:], in_=ident[:])
            if not mask_ones:
                nc.scalar.dma_start(out=mc_sb[:], in_=mcol[:])
            nc.sync.dma_start(
                out=wkqv_sb[:, 0:2, :, :],
                in_=wkq.rearrange("w (fo p) o -> p w fo o", p=P),
            )
            dma_x(xk_r, xk_sb, 0)
            dma_x(xq_r, xq_sb, 0)
            dma_x(xq_r, xq_sb, 1)
            nc.sync.dma_start(
                out=wkqv_sb[:, 2, :, :],
                in_=wvd.rearrange("(fo p) o -> p fo o", p=P),
            )
            dma_x(xv_r, xv_sb, 0)
            dma_x(xk_r, xk_sb, 1)
            dma_x(xv_r, xv_sb, 1)
            dma_x(xk_r, xk_sb, 2)
            dma_x(xv_r, xv_sb, 2)
            dma_x(xk_r, xk_sb, 3)
            dma_x(xv_r, xv_sb, 3)
            dma_x(xq_r, xq_sb, 2)
            dma_x(xq_r, xq_sb, 3)
            nc.sync.dma_start(out=wo_sb[:], in_=wo.rearrange("(oh p) f -> p oh f", p=P))

            # V2 ones column (or mask column)
            nc.vector.memset(V2[:, :, :, DK:D1], 1.0)
            if not mask_ones:
                nc.vector.memset(z3_sb[:], 0.0)

            # ---- projection emitters ----
            def qk_chunk(w_sb, b_col, dst, x_sb, po, c, lo=0, hi=F):
                w = hi - lo
                pp = ps_pp.tile([P, F], f32, tag="pp",
                                name=f"qk_{po}_{c}_{lo}")
                for fo in range(FO):
                    nc.tensor.matmul(
                        pp[:, 0:w],
                        w_sb[:, fo, po * P:(po + 1) * P],
                        x_sb[:, fo, c * F + lo:c * F + hi],
                        start=(fo == 0),
                        stop=(fo == FO - 1),
                    )
                nc.vector.tensor_scalar_add(
                    dst[:, po, c * F + lo:c * F + hi], pp[:, 0:w],
                    b_col[:, po:po + 1]
                )

            def v_chunk(tb):
                pp = ps_pp.tile([P, F], f32, tag="pp", name=f"v_{tb}")
                for fo in range(FO):
                    nc.tensor.matmul(
                        pp[:, :OB],
                        xv_sb[:, fo, tb * P:(tb + 1) * P],
                        wv_sb[:, fo, :],
                        start=(fo == 0),
                        stop=(fo == FO - 1),
                    )
                if mask_ones:
                    nc.vector.tensor_add(
                        V2[:, tb, :, 0:DK],
                        pp[:, :OB].rearrange("p (h d) -> p h d", h=NH),
                        bv_sb[:].rearrange("p (h d) -> p h d", h=NH),
                    )
                else:
                    vt = npool.tile([P, NH, DK], f32, tag="vt")
                    nc.vector.tensor_add(
                        vt[:],
                        pp[:, :OB].rearrange("p (h d) -> p h d", h=NH),
                        bv_sb[:].rearrange("p (h d) -> p h d", h=NH),
                    )
                    nc.vector.tensor_scalar_mul(
                        V2[:, tb, :, 0:DK], vt[:], mc_sb[:, tb:tb + 1]
                    )
                    nc.vector.tensor_scalar_add(
                        V2[:, tb, :, DK:D1], z3_sb[:], mc_sb[:, tb:tb + 1]
                    )

            # ---- head start: K po0 c0, Q po0 c0-c1 ----
            qk_chunk(wk_sb, bk_col, KT, xk_sb, 0, 0)
            qk_chunk(wq_sb, bq_col, QT, xq_sb, 0, 0)

            # ---- norm / transpose / outproj emitters ----
            def emit_norm_mul(xn, ib, src_ap, rz, eng_act=False):
                if eng_act:
                    nc.scalar.mul(xn[:, ib, :], src_ap, rz[:, ib:ib + 1])
                else:
                    nc.vector.tensor_scalar_mul(xn[:, ib, :], src_ap,
                                                rz[:, ib:ib + 1])

            xn2_hold = {}

            def get_xn2(oh, su):
                if (oh, su) not in xn2_hold:
                    xn2_hold[(oh, su)] = npool.tile(
                        [P, NIB, 2, DK], bf16, tag="xn",
                        name=f"xn2_{oh}_{su}")
                return xn2_hold[(oh, su)]

            def emit_norm(pi, h, su, xpa, xpb):
                zeps = npool.tile([P, NIB], f32, tag="zeps")
                za = xpa[:].rearrange("p (ib c) -> p ib c", c=D1)[
                    :, :, DK:D1].rearrange("p ib one -> p (ib one)")
                zb = xpb[:].rearrange("p (ib c) -> p ib c", c=D1)[
                    :, :, DK:D1].rearrange("p ib one -> p (ib one)")
                nc.vector.tensor_scalar_add(zeps[:, 0:4], za, EPS)
                nc.vector.tensor_scalar_add(zeps[:, 4:8], zb, EPS)
                rz = npool.tile([P, NIB], f32, tag="rz")
                nc.vector.reciprocal(rz[:], zeps[:])
                xn2 = get_xn2(h // 2, su)
                for ib in range(NIB):
                    src = xpa if ib < 4 else xpb
                    ib2 = ib % 4
                    nc.vector.tensor_scalar_mul(
                        xn2[:, ib, h % 2, :],
                        src[:, ib2 * D1:ib2 * D1 + DK],
                        rz[:, ib:ib + 1],
                    )
                return xn2

            def emit_tp_ib(tp, xn2, ib):
                # full 128x128 transpose: [i, (h d)] -> [(h d), i]
                nc.tensor.transpose(
                    tp[:, ib * P:(ib + 1) * P],
                    xn2[:, ib, :, :].rearrange("p h d -> p (h d)"),
                    id_sb[:],
                )

            def emit_tp(oh, su, xn2, ib_lo=0, nib=NIB):
                tp = ps_pp.tile([P, ISUP], bf16, tag="pp",
                                name=f"tp_{oh}_{su}_{ib_lo}")
                for ib in range(ib_lo, ib_lo + nib):
                    emit_tp_ib(tp, xn2, ib)
                nc.vector.tensor_copy(
                    xT2[:, oh, su * ISUP + ib_lo * P:
                        su * ISUP + (ib_lo + nib) * P],
                    tp[:, ib_lo * P:(ib_lo + nib) * P],
                )

            os2_hold = {}

            def op_group(m2, su, cc, eng_act=False, dma_each=False):
                if cc == 0:
                    os2_hold[m2] = opool.tile([P, 2, F], bf16, tag="os2",
                                              name=f"os2_{su}_{m2}")
                os2 = os2_hold[m2]
                c = su * 2 + cc
                pp = ps_pp.tile([P, F], f32, tag="pp", name=f"op_{m2}_{c}")
                for oh in range(2):
                    nc.tensor.matmul(
                        pp[:, :F],
                        wo_sb[:, oh, m2 * P:(m2 + 1) * P],
                        xT2[:, oh, c * F:(c + 1) * F],
                        start=(oh == 0),
                        stop=(oh == 1),
                    )
                if eng_act:
                    nc.scalar.copy(os2[:, cc, :], pp[:, :F])
                else:
                    nc.vector.tensor_copy(os2[:, cc, :], pp[:, :F])
                if dma_each:
                    nc.sync.dma_start(
                        out=outT[m2 * P:(m2 + 1) * P, c * F:(c + 1) * F],
                        in_=os2[:, cc, :],
                    )
                elif cc == 1:
                    nc.sync.dma_start(
                        out=outT[m2 * P:(m2 + 1) * P, su * ISUP:(su + 1) * ISUP],
                        in_=os2[:].rearrange("p c f -> p (c f)"),
                    )

            # ---- per-pair interleaved extras ----
            def K_(po, c):
                return lambda: qk_chunk(wk_sb, bk_col, KT, xk_sb, po, c)

            def Q_(po, c):
                return lambda: qk_chunk(wq_sb, bq_col, QT, xq_sb, po, c)

            def OP_(m2, su, cc):
                return lambda: op_group(m2, su, cc)

            extras_by_pair = {
                0: {1: [K_(0, 1)], 2: [K_(0, 2)], 3: [K_(0, 3)]},
                1: {9: [K_(1, 0)], 10: [K_(1, 1)], 11: [K_(1, 2)],
                    12: [K_(1, 3)], 13: [Q_(1, 0)], 14: [Q_(1, 1)]},
                2: {8: [Q_(1, 2)], 10: [Q_(1, 3)], 12: [Q_(0, 2)],
                    14: [Q_(0, 3)]},
                5: {3: [OP_(0, 0, 0)], 5: [OP_(0, 0, 1)], 7: [OP_(1, 0, 0)],
                    9: [OP_(1, 0, 1)]},
                6: {1: [OP_(2, 0, 0)], 3: [OP_(2, 0, 1)], 5: [OP_(3, 0, 0)],
                    7: [OP_(3, 0, 1)]},
            }

            # ---- attention pairs ----
            # PV for pair p: 8 ib-major runs (16 consecutive matmuls each)
            # interleaved into pair p+1's jt loop. Last pair: jt-halves.
            pairs = [(su, h) for su in range(NSUP) for h in range(NH)]
            LAST = len(pairs) - 1
            LOJT = 12
            LO_SCHED = [[0, 1], [2, 3], [4, 5], [6, 7]]
            state = {}

            def pv_run(p, ib, jt0=0, njt=JT, tgt_pair=None, fresh=True):
                ps = state[p]
                if tgt_pair is None:
                    if ib == 0 and fresh:
                        ps["xpa"] = ps_xp.tile([P, 4 * D1], f32, tag="xpa",
                                               name=f"xpa_{p}")
                        ps["xpb"] = ps_xp.tile([P, 4 * D1], f32, tag="xpb",
                                               name=f"xpb_{p}")
                    tgt = ps["xpa"] if ib < 4 else ps["xpb"]
                else:
                    tgt = tgt_pair[0] if ib < 4 else tgt_pair[1]
                ib2 = ib % 4
                hp = ps["h"]
                for jj in range(njt):
                    jt = jt0 + jj
                    nc.tensor.matmul(
                        tgt[:, ib2 * D1:(ib2 + 1) * D1],
                        ps["et"][jt][:, ib * P:(ib + 1) * P],
                        V2[:, jt, hp, :],
                        start=(jj == 0),
                        stop=(jj == njt - 1),
                    )

            def drain_pair(p):
                ps = state[p]
                h, su = ps["h"], ps["su"]
                xn2 = emit_norm(p, h, su, ps["xpa"], ps["xpb"])
                if h % 2 == 1:
                    emit_tp(h // 2, su, xn2)
                    del xn2_hold[(h // 2, su)]
                del state[p]

            for pi, (su, h) in enumerate(pairs):
                qoff = (h % 2) * DK
                qpo = h // 2
                isl = su * ISUP
                extras = extras_by_pair.get(pi, {})
                state[pi] = {"h": h, "su": su, "et": []}

                def scores(jt):
                    st = ps_st.tile([P, ISUP], f32, tag="st",
                                    name=f"st_{pi}_{jt}")
                    for c2 in range(ISUP // F):
                        nc.tensor.matmul(
                            st[:, c2 * F:(c2 + 1) * F],
                            KT[qoff:qoff + DK, qpo, jt * P:(jt + 1) * P],
                            QT[qoff:qoff + DK, qpo,
                               isl + c2 * F:isl + (c2 + 1) * F],
                            start=True,
                            stop=True,
                        )
                    return st

                H2 = F // 2

                def st0_part(st, lo, hi):
                    nc.tensor.matmul(
                        st[:, lo:hi],
                        KT[qoff:qoff + DK, qpo, 0:P],
                        QT[qoff:qoff + DK, qpo, isl + lo:isl + hi],
                        start=True, stop=True,
                    )

                if pi == 0:
                    # first scores tile built in halves, so the exp stream
                    # starts as soon as the first QT half lands
                    st_prev = ps_st.tile([P, ISUP], f32, tag="st",
                                         name="st_0_0")
                    st0_part(st_prev, 0, F)
                else:
                    st_prev = scores(0)
                for jt in range(JT):
                    et = epool.tile([P, ISUP], bf16, tag="et",
                                    name=f"et_{pi}_{jt}")
                    if pi == 0 and jt == 0:
                        nc.scalar.activation(et[:, 0:F], st_prev[:, 0:F],
                                             Exp, scale=0.125)
                        qk_chunk(wq_sb, bq_col, QT, xq_sb, 0, 1)
                        st0_part(st_prev, F, ISUP)
                        nc.scalar.activation(et[:, F:ISUP], st_prev[:, F:ISUP],
                                             Exp, scale=0.125)
                    else:
                        nc.scalar.activation(et[:], st_prev[:], Exp,
                                             scale=0.125)
                    state[pi]["et"].append(et)
                    if jt + 1 < JT:
                        st_prev = scores(jt + 1)
                    if pi == 0:
                        v_chunk(jt)
                    if pi > 0 and jt < NIB:
                        pv_run(pi - 1, jt)
                    if pi > 0 and jt == NIB:
                        drain_pair(pi - 1)
                    if pi == LAST and jt >= 12:
                        # lo runs (jts 0-11) spread over jts 12-15
                        for ibx in LO_SCHED[jt - 12]:
                            pv_run(LAST, ibx, jt0=0, njt=LOJT,
                                   fresh=(ibx == 0))
                    for fn in extras.get(jt, []):
                        fn()

            # ---- tail: last pair hi-half + norm/tp/outproj per i-half ----
            ps7 = state[LAST]
            h7, su7 = ps7["h"], ps7["su"]
            # lo halves -> SBUF (overlaps the hi runs; DVE can read only one
            # PSUM operand per tensor_tensor)
            lo_sb = npool.tile([P, NIB * D1], f32, tag="losb")
            nc.vector.tensor_copy(lo_sb[:, 0:4 * D1], ps7["xpa"][:])
            nc.vector.tensor_copy(lo_sb[:, 4 * D1:NIB * D1], ps7["xpb"][:])
            hia = ps_pp.tile([P, 4 * D1], f32, tag="pp", name="hia")
            hib = ps_pp.tile([P, 4 * D1], f32, tag="pp", name="hib")
            for ib in range(NIB):
                pv_run(LAST, ib, jt0=LOJT, njt=JT - LOJT,
                       tgt_pair=(hia, hib))

            def z_of(xp):
                return xp[:].rearrange("p (ib c) -> p ib c", c=D1)[
                    :, :, DK:D1].rearrange("p ib one -> p (ib one)")

            # norm: z = (z_lo + eps) + z_hi fused; muls split DVE/ACT
            zs = npool.tile([P, NIB], f32, tag="zeps")
            nc.vector.scalar_tensor_tensor(
                out=zs[:, 0:4], in0=z_of(lo_sb)[:, 0:4], scalar=EPS,
                in1=z_of(hia), op0=mybir.AluOpType.add,
                op1=mybir.AluOpType.add)
            nc.vector.scalar_tensor_tensor(
                out=zs[:, 4:8], in0=z_of(lo_sb)[:, 4:8], scalar=EPS,
                in1=z_of(hib), op0=mybir.AluOpType.add,
                op1=mybir.AluOpType.add)
            rz7 = npool.tile([P, NIB], f32, tag="rz")
            nc.vector.reciprocal(rz7[:], zs[:])
            xn27 = get_xn2(h7 // 2, su7)
            xs7 = npool.tile([P, NIB, DK], f32, tag="xs7")

            def norm_half(ib_lo):
                hi = hia if ib_lo < 4 else hib
                lo4 = lo_sb[:].rearrange("p (ib c) -> p ib c", c=D1)[
                    :, ib_lo:ib_lo + 4, 0:DK]
                hi4 = hi[:].rearrange("p (ib c) -> p ib c", c=D1)[
                    :, :, 0:DK]
                nc.vector.tensor_add(xs7[:, ib_lo:ib_lo + 4, :], lo4, hi4)
                for ib in range(ib_lo, ib_lo + 4):
                    if ib % 2 == 1:
                        nc.scalar.mul(xn27[:, ib, h7 % 2, :], xs7[:, ib, :],
                                      rz7[:, ib:ib + 1])
                    else:
                        nc.vector.tensor_scalar_mul(
                            xn27[:, ib, h7 % 2, :], xs7[:, ib, :],
                            rz7[:, ib:ib + 1])

            norm_half(0)
            norm_half(4)
            emit_tp(h7 // 2, su7, xn27, ib_lo=0, nib=4)
            for m2 in range(F // P):
                op_group(m2, 1, 0, eng_act=(m2 % 2 == 1), dma_each=True)
            emit_tp(h7 // 2, su7, xn27, ib_lo=4, nib=4)
            for m2 in range(F // P):
                op_group(m2, 1, 1, eng_act=(m2 % 2 == 1), dma_each=True)

    nc.compile()
    return nc


def _prep_in_maps(query, key, value, mask, Wq, bq, Wk, bk, Wv, bv, Wo,
                  mask_ones):
    ident = np.eye(P, dtype=np.float32).astype(BF)
    B = query.shape[0]
    xTs = {}
    for b in range(B):
        m01 = (mask[b, 0, :] != 0)
        xv_full = value[b] * m01[:, None].astype(np.float32)
        xTs[b] = (
            np.ascontiguousarray(key[b].T).astype(BF),
            np.ascontiguousarray(query[b].T).astype(BF),
            np.ascontiguousarray(xv_full.T).astype(BF),
            np.ascontiguousarray(
                m01.astype(np.float32).reshape(NT, P).T) if not mask_ones
            else None,
        )
    in_maps = []
    for c in range(8):
        b = c // 2
        hh = c % 2
        ob = slice(hh * OB, (hh + 1) * OB)
        xkT, xqT, xvT, mc = xTs[b]
        bqk_h = np.concatenate(
            [bk[ob].reshape(OB // P, P).T, bq[ob].reshape(OB // P, P).T],
            axis=1,
        )
        wkq_h = np.stack([
            np.ascontiguousarray(Wk[ob, :].T),
            np.ascontiguousarray(Wq[ob, :].T),
        ]).astype(BF)
        wv_h = np.ascontiguousarray(Wv[ob, :].T).astype(BF)
        m = {
            "xk": xkT,
            "xq": xqT,
            "xv": xvT,
            "wkq": wkq_h,
            "wvd": wv_h,
            "wo": np.ascontiguousarray(Wo[:, ob].T).astype(BF),
            "bqk": np.ascontiguousarray(bqk_h),
            "bvb": np.ascontiguousarray(np.tile(bv[ob][None, :], (P, 1))),
            "ident": ident,
        }
        if not mask_ones:
            m["mcol"] = mc
        in_maps.append(m)
    return in_maps


def kernel(query, key, value, mask, Wq, bq, Wk, bk, Wv, bv, Wo, bo):
    query = np.asarray(query, dtype=np.float32)
    key = np.asarray(key, dtype=np.float32)
    value = np.asarray(value, dtype=np.float32)
    mask = np.asarray(mask)
    Wq = np.asarray(Wq, dtype=np.float32)
    bq = np.asarray(bq, dtype=np.float32)
    Wk = np.asarray(Wk, dtype=np.float32)
    bk = np.asarray(bk, dtype=np.float32)
    Wv = np.asarray(Wv, dtype=np.float32)
    bv = np.asarray(bv, dtype=np.float32)
    Wo = np.asarray(Wo, dtype=np.float32)
    bo = np.asarray(bo, dtype=np.float32)

    mask_ones = bool(np.all(mask != 0))
    ckey = ("nc", mask_ones)
    if ckey not in _CACHE:
        _CACHE[ckey] = _build(mask_ones)
        _CACHE["nc"] = _CACHE[ckey]  # test.py reads _CACHE["nc"]
    nc = _CACHE[ckey]

    B = query.shape[0]
    in_maps = _prep_in_maps(
        query, key, value, mask, Wq, bq, Wk, bk, Wv, bv, Wo, mask_ones
    )
    res = None
    for attempt in range(3):
        try:
            res = run_bass_kernel_spmd(nc, in_maps, core_ids=list(range(8)))
            break
        except Exception:
            if attempt == 2:
                raise

    out = np.empty((B, T, F), dtype=np.float32)
    for b in range(B):
        acc = (np.asarray(res.results[2 * b]["outT"], dtype=np.float32)
               + np.asarray(res.results[2 * b + 1]["outT"], dtype=np.float32))
        out[b] = acc.T + bo[None, :]
    return out
